# revision 9
# baseline (speedup 1.0000x reference)
"""GQA kernel for Trainium2, 8 NeuronCores, group-per-core sharding.

Reference: B=2, S=2048, D=2048, H=32 heads, G=8 kv groups (GS=4, HD=64).
Core g owns kv group g (4 heads = 2 pairs). All SBUF data bf16; PSUM f32.
Host sums the 8 cores' bf16 partial outputs (contraction-sharded Wo) in f32.

Layouts (per core):
  QT[p][b]   [128, S] bf16: rows 0-63 = head 2p Q^T (hd-major), 64-127 = head 2p+1
  K2T[b]     [128, S] bf16: rows 0-63 = K^T, rows 64-127 = same K^T (copy for
             the row-tiled second matmul of a pair)
  vaug[b][kt][128, 65] bf16: V natural rows + ones col (softmax denominator)
  scores     pair tile [128, 1024] f32 psum (2 banks): head A cols 0-512,
             head B 512-1024. MM1 = two row-tiled matmuls (contraction rows
             0-63 / 64-127) that run concurrently on the PE sub-arrays.
  exp        one ACT instr per pair tile -> w bf16 SBUF [128, 1024]
  MM2        per head: out = vaug.T @ w -> ctx psum [65, 512] (row 64 = denom)
  normalize  DVE recip + ones-matmul broadcast + ACT copy + DVE mul -> ctxn bf16
  out proj   ctxn.T @ woT -> psum [128,512], copy (ACT/DVE alternating) ->
             osb bf16 -> DMA out (issued from the idle GPSIMD queue).

Schedule notes:
  - Projection chunks, attention, and output projection are interleaved per
    512-token chunk; each chunk's out-proj matmuls are DEFERRED and popped as
    PE filler inside the next chunk's attention loop so the tensor engine
    never idles (keeps the HAM clock at 2.4 GHz, avoids p-state re-throttle).
  - x is loaded as ONE batched 3D DMA per chunk (16 separate tile DMAs
    saturated the SP sequencer with descriptor generation).
  - PSUM budget exactly 8 banks: scores 2x2 + ctx 2 + transient pool 2.
Causal: only lower-triangular k-tiles; the 4 diagonal tiles per (pair,qi)
are q-restricted to [dg*128, 512) and get a [128,128] triangular mask mul.
"""
import numpy as np
import ml_dtypes

import concourse.bacc as bacc
import concourse.mybir as mybir
import concourse.tile as tile
from concourse.bass_utils import run_bass_kernel_spmd

F32 = mybir.dt.float32
F32R = mybir.dt.float32r
BF16 = mybir.dt.bfloat16
AF = mybir.ActivationFunctionType

B, S, D = 2, 2048, 2048
G, GS, HD = 8, 4, 64
T = B * S            # 4096 flattened tokens
QCH = 512            # q-chunk (psum free dim)
NQC = S // QCH       # 4 q-chunks per batch
NKT = S // 128       # 16 k-tiles per batch
NKD = D // 128       # 16 contraction tiles over D


def build_nc():
    nc = bacc.Bacc("TRN2", target_bir_lowering=False, debug=False)
    xT = nc.dram_tensor("xT", [D, T], BF16, kind="ExternalInput")
    wqT = nc.dram_tensor("wqT", [D, GS * HD], BF16, kind="ExternalInput")
    wkvT = nc.dram_tensor("wkvT", [D, 2 * HD], BF16, kind="ExternalInput")
    woT = nc.dram_tensor("woT", [GS * HD, D], BF16, kind="ExternalInput")
    masks = nc.dram_tensor("masks", [128, 256], BF16, kind="ExternalInput")
    auxb = nc.dram_tensor("auxb", [128, 128], BF16, kind="ExternalInput")
    auxr = nc.dram_tensor("auxr", [128, 64], F32R, kind="ExternalInput")
    outp = nc.dram_tensor("outp", [T, D], BF16, kind="ExternalOutput")

    with tile.TileContext(nc) as tc:
        with tc.tile_pool(name="const", bufs=1) as const, \
             tc.tile_pool(name="store", bufs=1) as store:
            # --- static tiles -------------------------------------------------
            wq_sb = const.tile([128, NKD, GS * HD], BF16)
            wqre = xT_re(wqT)
            nc.sync.dma_start(out=wq_sb[:, 0:4, :], in_=wqre[:, 0:4, :])
            wkv_sb = const.tile([128, NKD, 2 * HD], BF16)
            wkvre = xT_re(wkvT)
            wo_sb = [const.tile([128, D], BF16, tag=f"wo{p}", name=f"wo{p}")
                     for p in range(2)]
            mask_sb = const.tile([128, 256], BF16)
            auxb_sb = const.tile([128, 128], BF16)
            auxr_sb = const.tile([128, 64], F32R)
            nc.sync.dma_start(out=auxr_sb[:], in_=auxr[:])

            # long-lived activations
            QT = [[store.tile([128, S], BF16, tag=f"qt{p}{b}", name=f"qt{p}{b}")
                   for b in range(B)] for p in range(2)]
            K2T = [store.tile([128, S], BF16, tag=f"kt{b}", name=f"ktt{b}")
                   for b in range(B)]
            VT = [store.tile([64, S], BF16, tag=f"vt{b}", name=f"vtt{b}")
                  for b in range(B)]
            vaug = [[store.tile([128, HD + 1], BF16, tag=f"va{b}_{kt}",
                                name=f"va{b}_{kt}")
                     for kt in range(NKT)] for b in range(B)]

            xre = xT.rearrange("(kt p) t -> p kt t", p=128)

            with tc.tile_pool(name="xp", bufs=2) as xp, \
                 tc.tile_pool(name="wp", bufs=7) as wp, \
                 tc.tile_pool(name="rp", bufs=2) as rp, \
                 tc.tile_pool(name="cn", bufs=2) as cn, \
                 tc.tile_pool(name="ob", bufs=4) as ob, \
                 tc.tile_pool(name="pss", bufs=2, space="PSUM") as pss, \
                 tc.tile_pool(name="psc", bufs=2, space="PSUM") as psc, \
                 tc.tile_pool(name="pst", bufs=2, space="PSUM") as pst:
                ofill = []  # deferred oproj emission closures (prev chunk)
                for b in range(B):
                    for qi in range(NQC):
                        # ---- projection chunk (b, qi): tokens col..col+512
                        col = qi * QCH
                        tch = b * NQC + qi
                        xch = xp.tile([128, NKD, QCH], BF16, tag="xt", name=f"xt{tch}")
                        tcol = tch * QCH
                        if tch == 0:
                            # stream first-chunk x and wq in 2-tile slices so
                            # the projection chain starts at DMA pace
                            for j in range(0, 8, 2):
                                nc.sync.dma_start(
                                    out=xch[:, j:j + 2, :],
                                    in_=xre[:, j:j + 2, tcol:tcol + QCH])
                                nc.sync.dma_start(out=wq_sb[:, j + 4:j + 6, :],
                                                  in_=wqre[:, j + 4:j + 6, :])
                            nc.sync.dma_start(out=xch[:, 8:NKD, :],
                                              in_=xre[:, 8:NKD, tcol:tcol + QCH])
                            nc.sync.dma_start(out=wq_sb[:, 12:NKD, :],
                                              in_=wqre[:, 12:NKD, :])
                            nc.sync.dma_start(out=wkv_sb[:], in_=wkvre)
                            nc.sync.dma_start(out=mask_sb[:], in_=masks[:])
                            nc.sync.dma_start(out=auxb_sb[:], in_=auxb[:])
                            for p_ in range(2):
                                nc.sync.dma_start(out=wo_sb[p_][:],
                                                  in_=woT[p_ * 128:(p_ + 1) * 128, :])
                        else:
                            nc.sync.dma_start(out=xch[:],
                                              in_=xre[:, :, tcol:tcol + QCH])
                        ps_q2 = pss.tile([128, 2 * QCH], F32, tag="sc",
                                          name=f"psq{tch}")
                        for p in range(2):
                            for kt in range(NKD):
                                nc.tensor.matmul(
                                    ps_q2[:, p * QCH:(p + 1) * QCH],
                                    wq_sb[:, kt, p * 128:(p + 1) * 128],
                                    xch[:, kt, :], start=(kt == 0), stop=(kt == NKD - 1))
                            nc.scalar.activation(
                                QT[p][b][:, col:col + QCH],
                                ps_q2[:, p * QCH:(p + 1) * QCH], AF.Copy)
                        ps_kv = pss.tile([128, 2 * QCH], F32, tag="sc",
                                         name=f"pskv{tch}")
                        for kt in range(NKD):
                            nc.tensor.matmul(ps_kv[:, 0:QCH], wkv_sb[:, kt, :],
                                             xch[:, kt, :],
                                             start=(kt == 0), stop=(kt == NKD - 1))
                        nc.vector.tensor_copy(K2T[b][0:64, col:col + QCH], ps_kv[0:64, 0:QCH])
                        nc.vector.tensor_copy(K2T[b][64:128, col:col + QCH], ps_kv[0:64, 0:QCH])
                        nc.vector.tensor_copy(VT[b][:, col:col + QCH], ps_kv[64:128, 0:QCH])
                        # v transposes for the 4 new k-tiles
                        for j in range(4):
                            kt = qi * 4 + j
                            ps_t = pst.tile([128, HD], BF16, tag="tr")
                            nc.tensor.transpose(
                                ps_t[:], VT[b][:, kt * 128:(kt + 1) * 128],
                                auxb_sb[0:64, 0:64])
                            nc.vector.tensor_copy(vaug[b][kt][:, 0:HD], ps_t[:])
                            nc.vector.tensor_copy(vaug[b][kt][:, HD:HD + 1],
                                                  auxb_sb[:, 64:65])

                        # ---- attention for (b, qi), pair-sequential ----------
                        kmax = 4 * (qi + 1)
                        ctxn = [cn.tile([128, QCH], BF16, tag=f"cn{p}", name=f"cn{tch}_{p}")
                                for p in range(2)]
                        for p in range(2):
                            ctx = [psc.tile([HD + 1, QCH], F32, tag="ctx",
                                            name=f"ctx{tch}_{p}_{h}")
                                   for h in range(2)]
                            pend = []
                            for kt in range(kmax):
                                dg = kt - 4 * qi
                                q0 = dg * 128 if dg >= 0 else 0
                                qw = QCH - q0
                                ps_s = pss.tile([128, 2 * QCH], F32, tag="sc")
                                for h in range(2):
                                    r0 = h * 64
                                    nc.tensor.matmul(
                                        ps_s[:, h * QCH + q0:(h + 1) * QCH],
                                        K2T[b][r0:r0 + 64, kt * 128:(kt + 1) * 128],
                                        QT[p][b][r0:r0 + 64, col + q0:col + QCH],
                                        start=True, stop=True)
                                w = wp.tile([128, 2 * QCH], BF16, tag="w")
                                if q0:
                                    ps_v = ps_s[:].rearrange(
                                        "p (h q) -> p h q", h=2)[:, :, q0:QCH]
                                    w_v = w[:].rearrange(
                                        "p (h q) -> p h q", h=2)[:, :, q0:QCH]
                                    nc.scalar.activation(w_v, ps_v, AF.Exp, scale=0.125)
                                else:
                                    nc.scalar.activation(w[:], ps_s[:], AF.Exp,
                                                         scale=0.125)
                                if dg >= 0:
                                    wm = w[:].rearrange(
                                        "p (h q) -> p h q", h=2)[:, :, q0:q0 + 128]
                                    mk = mask_sb[:].rearrange(
                                        "p (h q) -> p h q", h=2)
                                    nc.vector.tensor_mul(wm, wm, mk)
                                pend.append((kt, q0, w))
                                if ofill:
                                    ofill.pop(0)()  # PE filler: keeps clock warm
                                if len(pend) > 4:
                                    mm2(nc, pend.pop(0), ctx, vaug[b], qi)
                            while pend:
                                mm2(nc, pend.pop(0), ctx, vaug[b], qi)
                            # epilogue: normalize both heads of the pair
                            rr = rp.tile([128, QCH], F32R, tag="rr")
                            for h in range(2):
                                row = 64 + 32 * h
                                with nc.allow_low_precision(reason="softmax recip"):
                                    nc.vector.reciprocal(
                                        rr[row:row + 1, :], ctx[h][64:65, :])
                            for h in range(2):
                                row = 64 + 32 * h
                                rbc = pst.tile([64, QCH], F32, tag="tr")
                                nc.tensor.matmul(
                                    rbc[:], auxr_sb[row:row + 1, :],
                                    rr[row:row + 1, :], start=True, stop=True,
                                    tile_position=(row, 0))
                                cs = rp.tile([64, QCH], F32, tag="cs")
                                nc.scalar.activation(cs[:], ctx[h][0:64, :], AF.Copy)
                                nc.vector.tensor_mul(
                                    ctxn[p][h * 64:(h + 1) * 64, :],
                                    cs[:], rbc[:])
                            for _ in range(2):
                                if ofill:
                                    ofill.pop(0)()
                        # drain leftovers, then defer this chunk's out proj
                        for i, f in enumerate(ofill):
                            f(act=(i % 2 == 0))
                        ofill = oproj_closures(nc, pst, ob, ctxn, wo_sb, outp, b, qi)
                        if tch == B * NQC - 1:
                            for i, f in enumerate(ofill):
                                f(act=(i % 2 == 0))
                            ofill = []
    nc.compile()
    return nc


def oproj_closures(nc, pst, ob, ctxn, wo_sb, outp, b, qi):
    """One closure per psum group (2 matmuls + copy) plus one per row DMA.
    Popped inside the next chunk's attention loop as PE filler work."""
    F32_ = F32
    fs = []
    state = {}

    def group(tt, oc):
        def f(act=False):
            if oc == 0:
                state[tt] = ob.tile([128, D], BF16, tag="osb",
                                    name=f"osb{b}_{qi}_{tt}")
            osb = state[tt]
            ps_o = pst.tile([128, 512], F32_, tag="tr",
                            name=f"pso{b}_{qi}_{tt}_{oc}")
            for p in range(2):
                nc.tensor.matmul(
                    ps_o[:], ctxn[p][:, tt * 128:(tt + 1) * 128],
                    wo_sb[p][:, oc * 512:(oc + 1) * 512],
                    start=(p == 0), stop=(p == 1))
            dst = osb[:, oc * 512:(oc + 1) * 512]
            if act:
                nc.scalar.activation(dst, ps_o[:], AF.Copy)
            else:
                nc.vector.tensor_copy(dst, ps_o[:])
        return f

    def rowdma(tt):
        def f(act=False):
            row = b * S + qi * QCH + tt * 128
            nc.gpsimd.dma_start(out=outp[row:row + 128, :], in_=state[tt][:])
        return f

    for tt in range(QCH // 128):
        for oc in range(D // 512):
            fs.append(group(tt, oc))
        fs.append(rowdma(tt))
    return fs


def mm2(nc, item, ctx, vaug_b, qi):
    kt, q0, w = item
    for h in range(2):
        nc.tensor.matmul(
            ctx[h][:, q0:], vaug_b[kt][:],
            w[:, h * QCH + q0:(h + 1) * QCH],
            start=(kt == 0), stop=(kt == 4 * (qi + 1) - 1))


def xT_re(t):
    return t.rearrange("(kt p) c -> p kt c", p=128)


def prep_inputs(x, Wq, Wk, Wv, Wo):
    bf = ml_dtypes.bfloat16
    xT = np.ascontiguousarray(x.reshape(T, D).T).astype(bf)
    km = np.arange(128)[:, None]
    qm = np.arange(128)[None, :]
    tri = (km <= qm).astype(np.float32)
    masks = np.concatenate([tri, tri], axis=1).astype(bf)
    auxb = np.zeros((128, 128), dtype=np.float32)
    auxb[:64, :64] = np.eye(64, dtype=np.float32)
    auxb[:, 64:128] = 1.0
    auxr = np.ones((128, 64), dtype=np.float32)
    in_maps = []
    for g in range(G):
        in_maps.append({
            "xT": xT,
            "wqT": np.ascontiguousarray(Wq[g * GS * HD:(g + 1) * GS * HD, :].T).astype(bf),
            "wkvT": np.ascontiguousarray(
                np.concatenate([Wk[g * HD:(g + 1) * HD, :],
                                Wv[g * HD:(g + 1) * HD, :]], axis=0).T).astype(bf),
            "woT": np.ascontiguousarray(Wo[:, g * GS * HD:(g + 1) * GS * HD].T).astype(bf),
            "masks": masks,
            "auxb": auxb.astype(bf),
            "auxr": auxr,
        })
    return in_maps


def kernel(x, Wq, Wk, Wv, Wo):
    in_maps = prep_inputs(np.asarray(x, np.float32), np.asarray(Wq, np.float32),
                          np.asarray(Wk, np.float32), np.asarray(Wv, np.float32),
                          np.asarray(Wo, np.float32))
    nc = build_nc()
    res = run_bass_kernel_spmd(nc, in_maps, list(range(G)))
    acc = np.zeros((T, D), dtype=np.float32)
    for g in range(G):
        acc += res.results[g]["outp"].astype(np.float32)
    return acc.reshape(B, S, D)


# revision 10
# speedup vs baseline: 1.1960x; 1.1960x over previous
"""GQA kernel for Trainium2, 8 NeuronCores, group-per-core sharding.

Reference: B=2, S=2048, D=2048, H=32 heads, G=8 kv groups (GS=4, HD=64).
Core g owns kv group g (4 heads = 2 pairs). All SBUF data bf16; PSUM f32.
Host sums the 8 cores' bf16 partial outputs (contraction-sharded Wo) in f32.

Layouts (per core):
  QT[p][b]   [128, S] bf16: rows 0-63 = head 2p Q^T (hd-major), 64-127 = head 2p+1
  K2T[b]     [128, S] bf16: rows 0-63 = K^T, rows 64-127 = same K^T (copy for
             the row-tiled second matmul of a pair)
  vaug[b][kt][128, 65] bf16: V natural rows + ones col (softmax denominator)
  scores     pair tile [128, 1024] f32 psum (2 banks): head A cols 0-512,
             head B 512-1024. MM1 = two row-tiled matmuls (contraction rows
             0-63 / 64-127) that run concurrently on the PE sub-arrays.
  exp        one ACT instr per pair tile -> w bf16 SBUF [128, 1024]
  MM2        per head: out = vaug.T @ w -> ctx psum [65, 512] (row 64 = denom)
  normalize  DVE recip + ones-matmul broadcast + ACT copy + DVE mul -> ctxn bf16
  out proj   ctxn.T @ woT -> psum [128,512], copy (ACT/DVE alternating) ->
             osb bf16 -> DMA out (issued from the idle GPSIMD queue).

Schedule notes:
  - Projection chunks, attention, and output projection are interleaved per
    512-token chunk; each chunk's out-proj matmuls are DEFERRED and popped as
    PE filler inside the next chunk's attention loop so the tensor engine
    never idles (keeps the HAM clock at 2.4 GHz, avoids p-state re-throttle).
  - x is loaded as ONE batched 3D DMA per chunk (16 separate tile DMAs
    saturated the SP sequencer with descriptor generation).
  - PSUM budget exactly 8 banks: scores 2x2 + ctx 2 + transient pool 2.
Causal: only lower-triangular k-tiles; the 4 diagonal tiles per (pair,qi)
are q-restricted to [dg*128, 512) and get a [128,128] triangular mask mul.
"""
import numpy as np
import ml_dtypes

import concourse.bacc as bacc
import concourse.mybir as mybir
import concourse.tile as tile
from concourse.bass_utils import run_bass_kernel_spmd

F32 = mybir.dt.float32
F32R = mybir.dt.float32r
BF16 = mybir.dt.bfloat16
AF = mybir.ActivationFunctionType

B, S, D = 2, 2048, 2048
G, GS, HD = 8, 4, 64
T = B * S            # 4096 flattened tokens
QCH = 512            # q-chunk (psum free dim)
NQC = S // QCH       # 4 q-chunks per batch
NKT = S // 128       # 16 k-tiles per batch
NKD = D // 128       # 16 contraction tiles over D


def build_nc():
    nc = bacc.Bacc("TRN2", target_bir_lowering=False, debug=False)
    xT = nc.dram_tensor("xT", [D, T], BF16, kind="ExternalInput")
    wqT = nc.dram_tensor("wqT", [D, GS * HD], BF16, kind="ExternalInput")
    wkvT = nc.dram_tensor("wkvT", [D, 2 * HD], BF16, kind="ExternalInput")
    woT = nc.dram_tensor("woT", [GS * HD, D], BF16, kind="ExternalInput")
    masks = nc.dram_tensor("masks", [128, 256], BF16, kind="ExternalInput")
    auxb = nc.dram_tensor("auxb", [128, 128], BF16, kind="ExternalInput")
    auxr = nc.dram_tensor("auxr", [128, 64], F32R, kind="ExternalInput")
    outp = nc.dram_tensor("outp", [T, D], BF16, kind="ExternalOutput")

    with tile.TileContext(nc) as tc:
        with tc.tile_pool(name="const", bufs=1) as const, \
             tc.tile_pool(name="store", bufs=1) as store:
            # --- static tiles -------------------------------------------------
            wq_sb = const.tile([128, NKD, GS * HD], BF16)
            wqre = xT_re(wqT)
            nc.sync.dma_start(out=wq_sb[:, 0:4, :], in_=wqre[:, 0:4, :])
            wkv_sb = const.tile([128, NKD, 2 * HD], BF16)
            wkvre = xT_re(wkvT)
            wo_sb = [const.tile([128, D], BF16, tag=f"wo{p}", name=f"wo{p}")
                     for p in range(2)]
            mask_sb = const.tile([128, 256], BF16)
            auxb_sb = const.tile([128, 128], BF16)
            auxr_sb = const.tile([128, 64], F32R)
            nc.sync.dma_start(out=auxr_sb[:], in_=auxr[:])

            # long-lived activations
            QT = [[store.tile([128, S], BF16, tag=f"qt{p}{b}", name=f"qt{p}{b}")
                   for b in range(B)] for p in range(2)]
            K2T = [store.tile([128, S], BF16, tag=f"kt{b}", name=f"ktt{b}")
                   for b in range(B)]
            VT = [store.tile([64, S], BF16, tag=f"vt{b}", name=f"vtt{b}")
                  for b in range(B)]
            vaug = [[store.tile([128, HD + 1], BF16, tag=f"va{b}_{kt}",
                                name=f"va{b}_{kt}")
                     for kt in range(NKT)] for b in range(B)]

            xre = xT.rearrange("(kt p) t -> p kt t", p=128)

            with tc.tile_pool(name="xp", bufs=3) as xp, \
                 tc.tile_pool(name="wp", bufs=7) as wp, \
                 tc.tile_pool(name="rp", bufs=3) as rp, \
                 tc.tile_pool(name="cn", bufs=3) as cn, \
                 tc.tile_pool(name="ob", bufs=4) as ob, \
                 tc.tile_pool(name="pss", bufs=2, space="PSUM") as pss, \
                 tc.tile_pool(name="psc", bufs=2, space="PSUM") as psc, \
                 tc.tile_pool(name="pst", bufs=2, space="PSUM") as pst:
                ofill = []  # deferred oproj emission closures (prev chunk)
                for b in range(B):
                    for qi in range(NQC):
                        # ---- projection chunk (b, qi): tokens col..col+512
                        col = qi * QCH
                        tch = b * NQC + qi
                        xch = xp.tile([128, NKD, QCH], BF16, tag="xt", name=f"xt{tch}")
                        tcol = tch * QCH
                        if tch == 0:
                            # stream first-chunk x and wq in 2-tile slices so
                            # the projection chain starts at DMA pace
                            for j in range(0, 8, 2):
                                nc.sync.dma_start(
                                    out=xch[:, j:j + 2, :],
                                    in_=xre[:, j:j + 2, tcol:tcol + QCH])
                                nc.sync.dma_start(out=wq_sb[:, j + 4:j + 6, :],
                                                  in_=wqre[:, j + 4:j + 6, :])
                            nc.sync.dma_start(out=xch[:, 8:NKD, :],
                                              in_=xre[:, 8:NKD, tcol:tcol + QCH])
                            nc.sync.dma_start(out=wq_sb[:, 12:NKD, :],
                                              in_=wqre[:, 12:NKD, :])
                            nc.sync.dma_start(out=wkv_sb[:, 0:8, :],
                                              in_=wkvre[:, 0:8, :])
                            nc.sync.dma_start(out=wkv_sb[:, 8:NKD, :],
                                              in_=wkvre[:, 8:NKD, :])
                            nc.sync.dma_start(out=mask_sb[:], in_=masks[:])
                            nc.sync.dma_start(out=auxb_sb[:], in_=auxb[:])
                            for p_ in range(2):
                                nc.sync.dma_start(out=wo_sb[p_][:],
                                                  in_=woT[p_ * 128:(p_ + 1) * 128, :])
                        else:
                            nc.sync.dma_start(out=xch[:],
                                              in_=xre[:, :, tcol:tcol + QCH])
                        ps_q2 = pss.tile([128, 2 * QCH], F32, tag="sc",
                                          name=f"psq{tch}")
                        for p in range(2):
                            for kt in range(NKD):
                                nc.tensor.matmul(
                                    ps_q2[:, p * QCH:(p + 1) * QCH],
                                    wq_sb[:, kt, p * 128:(p + 1) * 128],
                                    xch[:, kt, :], start=(kt == 0), stop=(kt == NKD - 1))
                            nc.scalar.activation(
                                QT[p][b][:, col:col + QCH],
                                ps_q2[:, p * QCH:(p + 1) * QCH], AF.Copy)
                        ps_kv = pss.tile([128, 2 * QCH], F32, tag="sc",
                                         name=f"pskv{tch}")
                        for kt in range(NKD):
                            nc.tensor.matmul(ps_kv[:, 0:QCH], wkv_sb[:, kt, :],
                                             xch[:, kt, :],
                                             start=(kt == 0), stop=(kt == NKD - 1))
                        nc.vector.tensor_copy(K2T[b][0:64, col:col + QCH], ps_kv[0:64, 0:QCH])
                        nc.vector.tensor_copy(K2T[b][64:128, col:col + QCH], ps_kv[0:64, 0:QCH])
                        nc.vector.tensor_copy(VT[b][:, col:col + QCH], ps_kv[64:128, 0:QCH])
                        # v transposes for the 4 new k-tiles
                        for j in range(4):
                            kt = qi * 4 + j
                            ps_t = pst.tile([128, HD], BF16, tag="tr")
                            nc.tensor.transpose(
                                ps_t[:], VT[b][:, kt * 128:(kt + 1) * 128],
                                auxb_sb[0:64, 0:64])
                            nc.vector.tensor_copy(vaug[b][kt][:, 0:HD], ps_t[:])
                            nc.vector.tensor_copy(vaug[b][kt][:, HD:HD + 1],
                                                  auxb_sb[:, 64:65])

                        # ---- attention for (b, qi), pair-sequential ----------
                        kmax = 4 * (qi + 1)
                        ctxn = [cn.tile([128, QCH], BF16, tag=f"cn{p}", name=f"cn{tch}_{p}")
                                for p in range(2)]
                        for p in range(2):
                            ctx = [psc.tile([HD + 1, QCH], F32, tag="ctx",
                                            name=f"ctx{tch}_{p}_{h}")
                                   for h in range(2)]
                            pend = []
                            for kt in range(kmax):
                                dg = kt - 4 * qi
                                q0 = dg * 128 if dg >= 0 else 0
                                qw = QCH - q0
                                ps_s = pss.tile([128, 2 * QCH], F32, tag="sc")
                                for h in range(2):
                                    r0 = h * 64
                                    nc.tensor.matmul(
                                        ps_s[:, h * QCH + q0:(h + 1) * QCH],
                                        K2T[b][r0:r0 + 64, kt * 128:(kt + 1) * 128],
                                        QT[p][b][r0:r0 + 64, col + q0:col + QCH],
                                        start=True, stop=True)
                                w = wp.tile([128, 2 * QCH], BF16, tag="w")
                                if q0:
                                    ps_v = ps_s[:].rearrange(
                                        "p (h q) -> p h q", h=2)[:, :, q0:QCH]
                                    w_v = w[:].rearrange(
                                        "p (h q) -> p h q", h=2)[:, :, q0:QCH]
                                    nc.scalar.activation(w_v, ps_v, AF.Exp, scale=0.125)
                                else:
                                    nc.scalar.activation(w[:], ps_s[:], AF.Exp,
                                                         scale=0.125)
                                if dg >= 0:
                                    wm = w[:].rearrange(
                                        "p (h q) -> p h q", h=2)[:, :, q0:q0 + 128]
                                    mk = mask_sb[:].rearrange(
                                        "p (h q) -> p h q", h=2)
                                    nc.vector.tensor_mul(wm, wm, mk)
                                pend.append((kt, q0, w))
                                if ofill:
                                    ofill.pop(0)()  # PE filler: keeps clock warm
                                if len(pend) > 4:
                                    mm2(nc, pend.pop(0), ctx, vaug[b], qi)
                            while pend:
                                mm2(nc, pend.pop(0), ctx, vaug[b], qi)
                            # epilogue: normalize both heads of the pair
                            rr = rp.tile([128, QCH], F32R, tag="rr")
                            for h in range(2):
                                row = 64 + 32 * h
                                with nc.allow_low_precision(reason="softmax recip"):
                                    nc.vector.reciprocal(
                                        rr[row:row + 1, :], ctx[h][64:65, :])
                            for h in range(2):
                                row = 64 + 32 * h
                                rbc = pst.tile([64, QCH], F32, tag="tr")
                                nc.tensor.matmul(
                                    rbc[:], auxr_sb[row:row + 1, :],
                                    rr[row:row + 1, :], start=True, stop=True,
                                    tile_position=(row, 0))
                                cs = rp.tile([64, QCH], F32, tag="cs")
                                nc.scalar.activation(cs[:], ctx[h][0:64, :], AF.Copy)
                                nc.vector.tensor_mul(
                                    ctxn[p][h * 64:(h + 1) * 64, :],
                                    cs[:], rbc[:])
                            for _ in range(2):
                                if ofill:
                                    ofill.pop(0)()
                        # drain leftovers, then defer this chunk's out proj
                        for i, f in enumerate(ofill):
                            f(act=(i % 2 == 0))
                        ofill = oproj_closures(nc, pst, ob, ctxn, wo_sb, outp, b, qi)
                        if tch == B * NQC - 1:
                            for i, f in enumerate(ofill):
                                f(act=(i % 2 == 0))
                            ofill = []
    nc.compile()
    return nc


def oproj_closures(nc, pst, ob, ctxn, wo_sb, outp, b, qi):
    """One closure per psum group (2 matmuls + copy) plus one per row DMA.
    Popped inside the next chunk's attention loop as PE filler work."""
    F32_ = F32
    fs = []
    state = {}

    def group(tt, oc):
        def f(act=False):
            if oc == 0:
                state[tt] = ob.tile([128, D], BF16, tag="osb",
                                    name=f"osb{b}_{qi}_{tt}")
            osb = state[tt]
            ps_o = pst.tile([128, 512], F32_, tag="tr",
                            name=f"pso{b}_{qi}_{tt}_{oc}")
            for p in range(2):
                nc.tensor.matmul(
                    ps_o[:], ctxn[p][:, tt * 128:(tt + 1) * 128],
                    wo_sb[p][:, oc * 512:(oc + 1) * 512],
                    start=(p == 0), stop=(p == 1))
            dst = osb[:, oc * 512:(oc + 1) * 512]
            if act:
                nc.scalar.activation(dst, ps_o[:], AF.Copy)
            else:
                nc.vector.tensor_copy(dst, ps_o[:])
        return f

    def rowdma(tt):
        def f(act=False):
            row = b * S + qi * QCH + tt * 128
            nc.gpsimd.dma_start(out=outp[row:row + 128, :], in_=state[tt][:])
        return f

    for tt in range(QCH // 128):
        for oc in range(D // 512):
            fs.append(group(tt, oc))
        fs.append(rowdma(tt))
    return fs


def mm2(nc, item, ctx, vaug_b, qi):
    kt, q0, w = item
    for h in range(2):
        nc.tensor.matmul(
            ctx[h][:, q0:], vaug_b[kt][:],
            w[:, h * QCH + q0:(h + 1) * QCH],
            start=(kt == 0), stop=(kt == 4 * (qi + 1) - 1))


def xT_re(t):
    return t.rearrange("(kt p) c -> p kt c", p=128)


def prep_inputs(x, Wq, Wk, Wv, Wo):
    bf = ml_dtypes.bfloat16
    xT = np.ascontiguousarray(x.reshape(T, D).T).astype(bf)
    km = np.arange(128)[:, None]
    qm = np.arange(128)[None, :]
    tri = (km <= qm).astype(np.float32)
    masks = np.concatenate([tri, tri], axis=1).astype(bf)
    auxb = np.zeros((128, 128), dtype=np.float32)
    auxb[:64, :64] = np.eye(64, dtype=np.float32)
    auxb[:, 64:128] = 1.0
    auxr = np.ones((128, 64), dtype=np.float32)
    in_maps = []
    for g in range(G):
        in_maps.append({
            "xT": xT,
            "wqT": np.ascontiguousarray(Wq[g * GS * HD:(g + 1) * GS * HD, :].T).astype(bf),
            "wkvT": np.ascontiguousarray(
                np.concatenate([Wk[g * HD:(g + 1) * HD, :],
                                Wv[g * HD:(g + 1) * HD, :]], axis=0).T).astype(bf),
            "woT": np.ascontiguousarray(Wo[:, g * GS * HD:(g + 1) * GS * HD].T).astype(bf),
            "masks": masks,
            "auxb": auxb.astype(bf),
            "auxr": auxr,
        })
    return in_maps


def kernel(x, Wq, Wk, Wv, Wo):
    in_maps = prep_inputs(np.asarray(x, np.float32), np.asarray(Wq, np.float32),
                          np.asarray(Wk, np.float32), np.asarray(Wv, np.float32),
                          np.asarray(Wo, np.float32))
    nc = build_nc()
    res = run_bass_kernel_spmd(nc, in_maps, list(range(G)))
    acc = np.zeros((T, D), dtype=np.float32)
    for g in range(G):
        acc += res.results[g]["outp"].astype(np.float32)
    return acc.reshape(B, S, D)
